# revision 19
# baseline (speedup 1.0000x reference)
"""Trainium2 Bass kernel for NodeReadout: out = relu(concat([node_feature, segment_sum(edge_state, edge_dst)]) @ W + b).

v4 strategy (8 NeuronCores, no collectives): PE-fused reduction+dense.
  - Shard edges by destination owner with a degree-balanced round-robin
    node deal; all cores run one NEFF (structure = per-degree-class max).
  - Edge stream in fp8 e3m4 (1 B/elem, halves HBM vs bf16; 4 mantissa
    bits keep quantization err ~1.3e-2 rel fro, under the 2e-2 gate).
    Column layout [128] = [feats of edge a (64) ; feats of edge b (64)].
  - The segment-sum NEVER materializes: accumulating matmuls with
    stationary bf16 weights compute the dense projection of the sum
    directly in PSUM:  psum[slab] = W1.T@nf_slab + sum_k [W2;W2].T@
    layer_k, where layer_k holds each node's k-th edge pair at that
    node's column. One rhs column per PE cycle (fp8e3 = 1-pass) ->
    ~0.42 ns/col; the DVE fold tree of v3 is eliminated entirely.
  - Nodes grouped by padded-even degree class; per class, 512-col slabs;
    slab-major layer order so each slab's PSUM chain is contiguous.
    nf matmuls hoisted per slab-group to amortize LDWEIGHTS (w1<->w22).
  - ACT applies bias+ReLU PSUM->SBUF bf16; host casts back to f32.
"""

import os
import sys
import types

import numpy as np

for _p in (
    "/root/.axon_site",
    "/root/.axon_site/_ro/trn_rl_repo",
    "/opt/trn_rl_repo",
):
    if os.path.isdir(_p) and _p not in sys.path:
        sys.path.append(_p)

import ml_dtypes

BF16 = ml_dtypes.bfloat16
E3M4 = ml_dtypes.float8_e3m4

N_CORES = 8
D = 64
SLAB = 512  # dense slab width (one PSUM bank of fp32)
PAD = 2  # degree padding multiple (columns hold edge pairs)
MIN_GROUP = int(os.environ.get("GNN_MINGROUP", str(64 * N_CORES)))
CHUNK_COLS = int(os.environ.get("GNN_CHUNK", "16384"))  # 2 MiB fp8 per chunk
GROUP_PAIRS = int(os.environ.get("GNN_GPAIRS", "2"))
PSUM_BUFS = 8

_last_exec_time_ns = None
_last_results = None


def _classes_and_deal(edge_dst, N):
    """Degree classes (even, rare ones merged upward) + round-robin deal."""
    deg = np.bincount(edge_dst, minlength=N)
    degp = np.maximum(PAD, (deg + PAD - 1) // PAD * PAD)
    vals, cnts = np.unique(degp, return_counts=True)
    classes = []
    run = 0
    for v, c in zip(vals, cnts):
        run += int(c)
        if run >= MIN_GROUP:
            classes.append(int(v))
            run = 0
    if run > 0 or not classes:
        classes.append(int(vals[-1]))
    cls = np.array(classes)
    degp = cls[np.searchsorted(cls, degp)]
    # nodes sorted by padded degree, dealt round-robin -> per-core
    # histograms match within 1
    rank = np.argsort(degp, kind="stable")
    core_nodes = [rank[c::N_CORES] for c in range(N_CORES)]
    return deg, degp, core_nodes


def _plan(degp):
    """Shared device work plan (column-tiled slab pairs).

    Slabs are paired (A, B); A's matmuls run at PE tile_position (0,0)
    (PSUM rows 0:64), B's at (0,64) (rows 64:128) — concurrent column
    groups double effective PE throughput for our 64-row outputs.

    Returns (cls_list, slabs, pairs, groups, units, chunks, NSLOT, E2,
    PCOLS):
      cls_list: [(d_c, n_g, s_off)]
      slabs:    [{cls, h, col0, sn, pair, half}]
      pairs:    [{a, b(|None), pcol0, pn}]
      groups:   [[pair idx]] (<= GROUP_PAIRS consecutive)
      units:    [{slab, k, sn, chunk, off, half}] in stream order
      chunks:   [{cols, eo}]
    """
    all_degs = sorted(int(v) for v in np.unique(degp))
    cls_list = []
    s_off = 0
    for d in all_degs:
        cnt = int(np.count_nonzero(degp == d))
        n = (cnt + N_CORES - 1) // N_CORES
        cls_list.append((d, n, s_off))
        s_off += n
    NSLOT = s_off

    slabs = []
    for ci, (d, n, so) in enumerate(cls_list):
        h = d // 2
        for s in range(0, n, SLAB):
            sn = min(SLAB, n - s)
            slabs.append(
                dict(cls=ci, h=h, col0=so + s, sn=sn, pair=None, half=0)
            )

    pairs = []
    pcol = 0
    for i in range(0, len(slabs), 2):
        a = i
        b = i + 1 if i + 1 < len(slabs) else None
        pn = max(slabs[a]["sn"], slabs[b]["sn"] if b is not None else 0)
        slabs[a]["pair"], slabs[a]["half"] = len(pairs), 0
        if b is not None:
            slabs[b]["pair"], slabs[b]["half"] = len(pairs), 1
        pairs.append(dict(a=a, b=b, pcol0=pcol, pn=pn))
        pcol += pn
    PCOLS = pcol

    groups = [
        list(range(g, min(g + GROUP_PAIRS, len(pairs))))
        for g in range(0, len(pairs), GROUP_PAIRS)
    ]

    units = []
    chunks = []
    cc = 0  # cols in current chunk
    eo = 0
    # taper chunk sizes at both ends: small head chunks let the PE start
    # ~1us in; small tail chunks shrink the post-stream PE catch-up
    sizes = [2048, 4096, 8192]
    T = 0
    for pr in pairs:
        for si in (pr["a"], pr["b"]):
            if si is not None:
                T += slabs[si]["h"] * slabs[si]["sn"]
        T += pr["pn"]

    def cap():
        i = len(chunks) - 1
        head = sizes[i] if 0 <= i < len(sizes) else CHUNK_COLS
        rem = T - eo
        if rem > 28672:
            tail = CHUNK_COLS
        elif rem > 12288:
            tail = 8192
        elif rem > 6144:
            tail = 4096
        else:
            tail = 2048
        return min(head, tail)

    def push(u, sn):
        nonlocal cc, eo
        if not chunks or cc + sn > cap():
            chunks.append(dict(cols=0, eo=eo))
            cc = 0
        u.update(chunk=len(chunks) - 1, off=cc)
        units.append(u)
        cc += sn
        chunks[-1]["cols"] += sn
        eo += sn

    for grp in groups:
        for pi in grp:
            pr = pairs[pi]
            sls = [pr["a"]] + ([pr["b"]] if pr["b"] is not None else [])
            hmax = max(slabs[s]["h"] for s in sls)
            for k in range(hmax):
                for si in sls:
                    sl = slabs[si]
                    if k >= sl["h"]:
                        continue
                    push(dict(kind="e", slab=si, k=k, sn=sl["sn"]), sl["sn"])
            # the pair's nf columns ride the stream right behind its
            # layers: rows 0:64 = nf of A's slots, 64:128 = B's
            push(dict(kind="nf", pair=pi, sn=pr["pn"]), pr["pn"])
    E2 = eo
    return cls_list, slabs, pairs, groups, units, chunks, NSLOT, E2, PCOLS


def _prepare(node_feature, edge_state, edge_dst, W, b):
    node_feature = np.ascontiguousarray(np.asarray(node_feature), dtype=np.float32)
    edge_state = np.ascontiguousarray(np.asarray(edge_state), dtype=np.float32)
    edge_dst = np.asarray(edge_dst).astype(np.int64)
    W = np.ascontiguousarray(np.asarray(W), dtype=np.float32)
    b = np.asarray(b, dtype=np.float32).reshape(D, 1)

    N = node_feature.shape[0]
    eid_sorted = np.argsort(edge_dst, kind="stable")
    deg, degp, core_nodes = _classes_and_deal(edge_dst, N)
    starts = np.cumsum(deg) - deg
    cls_list, slabs, pairs, groups, units, chunks, NSLOT, E2, PCOLS = _plan(degp)

    # out_t rows 0:64 hold pair half A, 64:128 half B; out_slot maps
    # (half, pair col) -> node slot
    out_slot = np.full((2, PCOLS), -1, dtype=np.int64)
    for pr in pairs:
        a = slabs[pr["a"]]
        out_slot[0, pr["pcol0"] : pr["pcol0"] + a["sn"]] = a["col0"] + np.arange(
            a["sn"]
        )
        if pr["b"] is not None:
            bsl = slabs[pr["b"]]
            out_slot[1, pr["pcol0"] : pr["pcol0"] + bsl["sn"]] = bsl[
                "col0"
            ] + np.arange(bsl["sn"])

    es8 = edge_state.astype(E3M4)
    es8 = np.concatenate([es8, np.zeros((1, D), dtype=E3M4)], axis=0)
    nf8 = node_feature.astype(E3M4)

    in_maps = []
    col_node = np.full((N_CORES, NSLOT), -1, dtype=np.int64)
    W8 = W.astype(BF16)
    for c in range(N_CORES):
        nodes = np.asarray(core_nodes[c])
        ndeg = degp[nodes]
        ems = {}
        for ci, (d, n_g, so) in enumerate(cls_list):
            nodes_d = nodes[ndeg == d]
            kk = len(nodes_d)
            em = np.full((n_g, d), -1, dtype=np.int64)
            if kk:
                col = starts[nodes_d][:, None] + np.arange(d)[None, :]
                valid = np.arange(d)[None, :] < deg[nodes_d][:, None]
                em[:kk] = np.where(
                    valid, eid_sorted[np.where(valid, col, 0)], -1
                )
                col_node[c, so : so + kk] = nodes_d
            ems[ci] = em
        gidx = np.full((2, E2), -1, dtype=np.int64)
        nf_gidx = np.full((2, E2), -1, dtype=np.int64)  # -1 -> zero row
        run = 0
        for u in units:
            sn = u["sn"]
            if u["kind"] == "e":
                sl = slabs[u["slab"]]
                em = ems[sl["cls"]]
                so = cls_list[sl["cls"]][2]
                s = sl["col0"] - so
                k = u["k"]
                gidx[0, run : run + sn] = em[s : s + sn, 2 * k]
                gidx[1, run : run + sn] = em[s : s + sn, 2 * k + 1]
            else:
                pr = pairs[u["pair"]]
                for half, si in ((0, pr["a"]), (1, pr["b"])):
                    if si is None:
                        continue
                    sl = slabs[si]
                    nf_gidx[half, run : run + sl["sn"]] = col_node[
                        c, sl["col0"] : sl["col0"] + sl["sn"]
                    ]
            run += sn
        edge_t = np.empty((2 * D, E2), dtype=E3M4)
        edge_t[0:D] = es8[gidx[0]].T
        edge_t[D : 2 * D] = es8[gidx[1]].T
        nf8x = np.concatenate([nf8, np.zeros((1, D), dtype=E3M4)], axis=0)
        for half in range(2):
            sel = np.nonzero(nf_gidx[half] != -1)[0]
            edge_t[half * D : half * D + D, sel] = nf8x[nf_gidx[half][sel]].T
        consts = np.empty((128, 129), dtype=BF16)
        consts[0:64, 0:64] = W8[0:64]
        consts[64:128, 0:64] = W8[0:64]
        consts[0:64, 64:128] = W8[64:128]
        consts[64:128, 64:128] = W8[64:128]
        consts[0:64, 128] = b[:, 0].astype(BF16)
        consts[64:128, 128] = b[:, 0].astype(BF16)
        in_maps.append(
            {
                "edge_t": np.ascontiguousarray(edge_t),
                "consts": consts,
            }
        )
    return (
        in_maps,
        (cls_list, slabs, pairs, groups, units, chunks),
        NSLOT,
        E2,
        PCOLS,
        col_node,
        out_slot,
        N,
    )


def _install_shims():
    """Environment fixes: antenv.axon_hooks shim (NTFF profiling), no-op
    artifact upload, and a TileContext drain patch (this container's walrus
    rejects >1 sync-wait per instruction)."""
    try:
        import antenv.axon_hooks  # noqa: F401
    except ImportError:
        try:
            import antenv

            mod = types.ModuleType("antenv.axon_hooks")
            mod._hook = None

            def set_axon_ntff_profile_hook(h):
                mod._hook = h

            def get_axon_ntff_profile_hook():
                return mod._hook

            mod.set_axon_ntff_profile_hook = set_axon_ntff_profile_hook
            mod.get_axon_ntff_profile_hook = get_axon_ntff_profile_hook
            sys.modules["antenv.axon_hooks"] = mod
            antenv.axon_hooks = mod
            try:
                from trn_agent_boot.trn_boot import _ntff_profile_via_ctypes

                so = "/opt/axon/libaxon_pjrt.so"
                if os.path.exists(so):
                    set_axon_ntff_profile_hook(_ntff_profile_via_ctypes(so))
            except Exception:
                pass
        except Exception:
            pass
    try:
        import concourse.bass_utils as bu

        bu.upload_artifacts = lambda tmpdir: "local://" + tmpdir
    except Exception:
        pass
    import concourse.mybir as mybir
    import concourse.tile as tile_mod
    from concourse.vector_clock import ScopedClock

    if getattr(tile_mod.TileContext, "_drain_patched", False):
        return
    tile_mod.TileContext._orig_drain_and_barrier = (
        tile_mod.TileContext._drain_and_barrier
    )

    def _drain_and_barrier(self, tick_clock, wait_clock):
        nc = self.nc
        probe = nc.sync.nop(nofuse=True, hint="drain_wait_split")
        wait_clock.add_sem_waits(
            probe.ins, ScopedClock({None: tick_clock.global_clock})
        )
        waits = list(probe.ins.sync_info.on_wait)
        probe.ins.sync_info.on_wait = waits[:1]
        for w in waits[1:]:
            nop = nc.sync.nop(nofuse=True, hint="drain_wait_split")
            nop.ins.sync_info = mybir.SyncInfo(on_update=[], on_wait=[w])
        nc.sync.drain()
        nc.all_engine_barrier()
        assert self.sems is not None
        popped = nc._tile_sem_poison_stack.pop()
        assert popped is self._sem_poison
        nc.clear_and_free_semaphores(list(self.sems.allocated().values()))
        nc.all_engine_barrier()

    tile_mod.TileContext._drain_and_barrier = _drain_and_barrier
    tile_mod.TileContext._patched_drain_and_barrier = _drain_and_barrier
    tile_mod.TileContext._drain_patched = True


def _split_multiwaits(nc):
    """Walrus here allows at most ONE sync-wait per instruction: hoist extra
    waits onto preceding NoOps on the same engine."""
    import concourse.mybir as mybir

    for fn in nc.m.functions:
        for blk in fn.blocks:
            insts = blk.instructions
            new = []
            for ins in insts:
                si = getattr(ins, "sync_info", None)
                waits = list(si.on_wait) if si is not None and si.on_wait else []
                if len(waits) > 1:
                    for jw, w in enumerate(waits[:-1]):
                        nop = mybir.InstNoOp(
                            name=f"{ins.name}-wsplit{jw}",
                            engine=ins.engine,
                            bass_nofuse=True,
                            sync_info=mybir.SyncInfo(on_update=[], on_wait=[w]),
                        )
                        new.append(nop)
                    si.on_wait = [waits[-1]]
                new.append(ins)
            blk.instructions[:] = new


def _prebarrier_dma_hoist(nc, per_engine=2):
    """Move each HWDGE engine's first wait-free DMACopys to before its
    entry-barrier depart (EventSemaphore): the transfers then stream
    during the ~7us NEFF/Tile preamble instead of after it. DMAHW sems
    are NRT-zeroed at load, so pre-barrier +16 updates are safe."""
    import concourse.mybir as mybir

    for fn in nc.m.functions:
        seq = []  # (blk, idx, ins) in program order
        for blk in fn.blocks:
            for i, ins in enumerate(blk.instructions):
                seq.append((blk, i, ins))
        for eng in ("SP", "Activation"):
            bar = None
            moves = []
            for blk, i, ins in seq:
                if str(ins.engine).split(".")[-1] != eng:
                    continue
                if bar is None:
                    if isinstance(
                        ins, mybir.InstEventSemaphore
                    ) and ins.name.startswith("barrier_"):
                        bar = (blk, ins)
                    continue
                if len(moves) >= per_engine:
                    break
                si = getattr(ins, "sync_info", None)
                if isinstance(ins, mybir.InstDMACopy) and not (
                    si is not None and si.on_wait
                ):
                    moves.append((blk, ins))
            if bar is None:
                continue
            bblk, bins = bar
            for mblk, mins in moves:
                mblk.instructions.remove(mins)
                bi = bblk.instructions.index(bins)
                bblk.instructions.insert(bi, mins)


def _build(plan, NSLOT, E2, PCOLS):
    import concourse.bass as bass
    import concourse.mybir as mybir
    from concourse.tile import TileContext

    cls_list, slabs, pairs, groups, units, chunks = plan
    f32 = mybir.dt.float32
    bf16 = mybir.dt.bfloat16
    f8e3 = mybir.dt.float8e3
    nc = bass.Bass("TRN2", target_bir_lowering=False, debug=False)
    edge_t = nc.declare_dram_parameter("edge_t", [128, E2], f8e3, isOutput=False)
    cp = nc.declare_dram_parameter("consts", [128, 129], bf16, isOutput=False)
    out_t = nc.declare_dram_parameter("out_t", [128, PCOLS], bf16, isOutput=True)

    # units grouped by pair, preserving stream (interleaved) order
    pair_units = {}
    for u in units:
        pi = u["pair"] if u["kind"] == "nf" else slabs[u["slab"]]["pair"]
        pair_units.setdefault(pi, []).append(u)

    with TileContext(nc) as tc:
        with (
            tc.tile_pool(name="const", bufs=1) as cpool,
            tc.tile_pool(name="edges", bufs=len(chunks)) as epool,
            tc.tile_pool(name="psum", bufs=PSUM_BUFS, space="PSUM") as ppool,
            tc.tile_pool(name="outs", bufs=1) as opool,
        ):
            ob_all = opool.tile([128, PCOLS], bf16)
            # edge chunk 0 first in sync's queue: the PE's first real work
            ech0 = epool.tile(
                [128, chunks[0]["cols"]], f8e3, tag="eb0", name="ebuf0", bufs=1
            )
            nc.sync.dma_start(
                out=ech0[:], in_=edge_t[:, 0 : chunks[0]["cols"]]
            )
            cb = cpool.tile([128, 129], bf16)
            nc.scalar.dma_start(out=cb[:], in_=cp[:])
            w1 = cb[:, 0:64]
            w22 = cb[:, 64:128]
            bt = cb[:, 128:129]

            # hoist every chunk DMA to the program head: per-chunk tiles
            # have no input deps, and issuing early keeps the HWDGE rings
            # streaming instead of starving behind ReLU work on ACT
            ebufs = {0: ech0}  # chunk idx -> tile
            for cidx in range(1, len(chunks)):
                ch = chunks[cidx]
                t = epool.tile(
                    [128, ch["cols"]], f8e3, tag=f"eb{cidx}",
                    name=f"ebuf{cidx}", bufs=1,
                )
                eng = nc.sync if (cidx <= 1 or cidx % 2 == 0) else nc.scalar
                eng.dma_start(
                    out=t[:], in_=edge_t[:, ch["eo"] : ch["eo"] + ch["cols"]]
                )
                ebufs[cidx] = t

            def get_ebuf(cidx):
                return ebufs[cidx]

            def mm(ps, sl, rhs, lhsT, start, stop):
                half = sl["half"]
                nc.tensor.matmul(
                    out=ps[64 * half : 64 * half + 64, : sl["sn"]],
                    lhsT=lhsT, rhs=rhs, start=start, stop=stop,
                    tile_position=(0, 64 * half),
                )

            st_i = 0
            for grp in groups:
                pst = {}
                for pi in grp:
                    ps = ppool.tile(
                        [128, SLAB], f32, space="PSUM", tag="ps", name=f"ps{pi}"
                    )
                    pst[pi] = ps
                    for u in pair_units[pi]:
                        eb = get_ebuf(u["chunk"])
                        o = u["off"]
                        if u["kind"] == "e":
                            sl = slabs[u["slab"]]
                            mm(
                                ps, sl, eb[:, o : o + u["sn"]],
                                w22, u["k"] == 0, False,
                            )
                        else:
                            pr = pairs[u["pair"]]
                            for half, si in ((0, pr["a"]), (1, pr["b"])):
                                if si is None:
                                    continue
                                sl = slabs[si]
                                nc.tensor.matmul(
                                    out=ps[
                                        64 * half : 64 * half + 64, : sl["sn"]
                                    ],
                                    lhsT=w1[64 * half : 64 * half + 64, :],
                                    rhs=eb[
                                        64 * half : 64 * half + 64,
                                        o : o + sl["sn"],
                                    ],
                                    start=False, stop=True,
                                    tile_position=(64 * half, 64 * half),
                                )
                for pi in grp:
                    pr = pairs[pi]
                    ps = pst.pop(pi)
                    pn = pr["pn"]
                    prows = 128 if pr["b"] is not None else 64
                    pc0 = pr["pcol0"]
                    nc.scalar.activation(
                        out=ob_all[:prows, pc0 : pc0 + pn],
                        in_=ps[:prows, :pn],
                        func=mybir.ActivationFunctionType.Relu,
                        bias=bt[:prows, :],
                    )
                # store this group's finished region on alternating rings
                g0 = pairs[grp[0]]["pcol0"]
                ge = pairs[grp[-1]]["pcol0"] + pairs[grp[-1]]["pn"]
                eng = nc.scalar if st_i % 2 == 0 else nc.sync
                st_i += 1
                eng.dma_start(
                    out=out_t[:, g0:ge], in_=ob_all[:, g0:ge]
                )
    _split_multiwaits(nc)
    if os.environ.get("GNN_PREBAR", "1") == "1":
        _prebarrier_dma_hoist(nc)
    return nc


def kernel(node_feature, edge_state, edge_dst, W, b):
    global _last_exec_time_ns, _last_results
    _install_shims()
    from concourse.bass_utils import run_bass_kernel_spmd

    in_maps, plan, NSLOT, E2, PCOLS, col_node, out_slot, N = _prepare(
        node_feature, edge_state, edge_dst, W, b
    )
    nc = _build(plan, NSLOT, E2, PCOLS)
    trace = bool(os.environ.get("GNN_TRACE"))
    res = run_bass_kernel_spmd(
        nc, in_maps, core_ids=list(range(N_CORES)), trace=trace
    )
    _last_exec_time_ns = res.exec_time_ns
    _last_results = res
    out = np.zeros((N, D), dtype=np.float32)
    for c in range(N_CORES):
        ot = np.asarray(res.results[c]["out_t"]).astype(np.float32)
        for half in range(2):
            pm = out_slot[half] >= 0
            slots = out_slot[half][pm]
            nodes = col_node[c][slots]
            v2 = nodes >= 0
            out[nodes[v2]] = ot[64 * half : 64 * half + 64][:, pm][:, v2].T
    return out


def last_exec_time_ns():
    return _last_exec_time_ns


def last_results():
    return _last_results


# revision 21
# speedup vs baseline: 1.0870x; 1.0870x over previous
"""Trainium2 Bass kernel for NodeReadout: out = relu(concat([node_feature, segment_sum(edge_state, edge_dst)]) @ W + b).

v11 strategy (8 NeuronCores, no collectives): PE-fused reduction+dense
with column-tiled pair streams. 97.2us baseline -> ~64us.
  - Shard edges by destination owner with a degree-balanced round-robin
    node deal; all cores run one NEFF (structure = per-degree-class max).
  - Edge stream in fp8 e3m4 (1 B/elem, halves HBM vs bf16; 4 mantissa
    bits keep total err ~1.36e-2 rel fro, under the 2e-2 gate).
    Column layout [128] = [feats of edge a (64) ; feats of edge b (64)].
  - The segment-sum NEVER materializes: accumulating matmuls with
    stationary bf16 weights compute the dense projection of the sum
    directly in PSUM: psum = W1.T@nf + sum_k [W2;W2].T@layer_k, where
    layer_k holds each node's k-th edge pair at that node's column.
    The v3 DVE fold tree is eliminated entirely (DVE idle).
  - Column tiling: slabs are paired; pair halves run concurrently at PE
    tile_position (0,0)/(0,64) into one [128,512] PSUM tile -> 2 rhs
    cols/cycle effective (~133 ns per 512-col matmul), PE busy ~27us.
  - Node features ride IN the stream: each pair appends one unit whose
    rows 0:64/64:128 are the pair halves' nf columns, closed by two
    concurrent w1 matmuls at tile (0,0)/(64,64). No separate nf DMA.
  - All chunk DMAs issue at the program head on alternating HWDGE rings
    (sync/scalar; ~15 MB resident SBUF) so the rings stream at full
    rate; graduated head chunk sizes (2k/4k/8k cols) start the PE ~1us
    after the preamble. All consts ship as one packed [128,129] DMA.
    gpsimd/SWDGE is never used (its ucode preamble stalls the barrier).
  - ACT applies bias+ReLU PSUM->SBUF bf16 per pair into one [128,PCOLS]
    output tile, stored per 2-pair group; host unshards and casts f32.
"""

import os
import sys
import types

import numpy as np

for _p in (
    "/root/.axon_site",
    "/root/.axon_site/_ro/trn_rl_repo",
    "/opt/trn_rl_repo",
):
    if os.path.isdir(_p) and _p not in sys.path:
        sys.path.append(_p)

import ml_dtypes

BF16 = ml_dtypes.bfloat16
E3M4 = ml_dtypes.float8_e3m4

N_CORES = 8
D = 64
SLAB = 512  # dense slab width (one PSUM bank of fp32)
PAD = 2  # degree padding multiple (columns hold edge pairs)
MIN_GROUP = int(os.environ.get("GNN_MINGROUP", str(64 * N_CORES)))
CHUNK_COLS = int(os.environ.get("GNN_CHUNK", "16384"))  # 2 MiB fp8 per chunk
GROUP_PAIRS = int(os.environ.get("GNN_GPAIRS", "2"))
PSUM_BUFS = 8

_last_exec_time_ns = None
_last_results = None


def _classes_and_deal(edge_dst, N):
    """Degree classes (even, rare ones merged upward) + round-robin deal."""
    deg = np.bincount(edge_dst, minlength=N)
    degp = np.maximum(PAD, (deg + PAD - 1) // PAD * PAD)
    vals, cnts = np.unique(degp, return_counts=True)
    classes = []
    run = 0
    for v, c in zip(vals, cnts):
        run += int(c)
        if run >= MIN_GROUP:
            classes.append(int(v))
            run = 0
    if run > 0 or not classes:
        classes.append(int(vals[-1]))
    cls = np.array(classes)
    degp = cls[np.searchsorted(cls, degp)]
    # nodes sorted by padded degree, dealt round-robin -> per-core
    # histograms match within 1
    rank = np.argsort(degp, kind="stable")
    core_nodes = [rank[c::N_CORES] for c in range(N_CORES)]
    return deg, degp, core_nodes


def _plan(degp):
    """Shared device work plan (column-tiled slab pairs).

    Slabs are paired (A, B); A's matmuls run at PE tile_position (0,0)
    (PSUM rows 0:64), B's at (0,64) (rows 64:128) — concurrent column
    groups double effective PE throughput for our 64-row outputs.

    Returns (cls_list, slabs, pairs, groups, units, chunks, NSLOT, E2,
    PCOLS):
      cls_list: [(d_c, n_g, s_off)]
      slabs:    [{cls, h, col0, sn, pair, half}]
      pairs:    [{a, b(|None), pcol0, pn}]
      groups:   [[pair idx]] (<= GROUP_PAIRS consecutive)
      units:    [{slab, k, sn, chunk, off, half}] in stream order
      chunks:   [{cols, eo}]
    """
    all_degs = sorted(int(v) for v in np.unique(degp))
    cls_list = []
    s_off = 0
    for d in all_degs:
        cnt = int(np.count_nonzero(degp == d))
        n = (cnt + N_CORES - 1) // N_CORES
        cls_list.append((d, n, s_off))
        s_off += n
    NSLOT = s_off

    slabs = []
    for ci, (d, n, so) in enumerate(cls_list):
        h = d // 2
        for s in range(0, n, SLAB):
            sn = min(SLAB, n - s)
            slabs.append(
                dict(cls=ci, h=h, col0=so + s, sn=sn, pair=None, half=0)
            )

    pairs = []
    pcol = 0
    for i in range(0, len(slabs), 2):
        a = i
        b = i + 1 if i + 1 < len(slabs) else None
        pn = max(slabs[a]["sn"], slabs[b]["sn"] if b is not None else 0)
        slabs[a]["pair"], slabs[a]["half"] = len(pairs), 0
        if b is not None:
            slabs[b]["pair"], slabs[b]["half"] = len(pairs), 1
        pairs.append(dict(a=a, b=b, pcol0=pcol, pn=pn))
        pcol += pn
    PCOLS = pcol

    groups = [
        list(range(g, min(g + GROUP_PAIRS, len(pairs))))
        for g in range(0, len(pairs), GROUP_PAIRS)
    ]

    units = []
    chunks = []
    cc = 0  # cols in current chunk
    eo = 0
    # small leading chunks let the PE start ~1us in instead of waiting
    # for a full 2 MiB transfer
    sizes = [2048, 4096, 8192]

    def cap():
        i = len(chunks) - 1
        return sizes[i] if 0 <= i < len(sizes) else CHUNK_COLS

    def push(u, sn):
        nonlocal cc, eo
        if not chunks or cc + sn > cap():
            chunks.append(dict(cols=0, eo=eo))
            cc = 0
        u.update(chunk=len(chunks) - 1, off=cc)
        units.append(u)
        cc += sn
        chunks[-1]["cols"] += sn
        eo += sn

    for grp in groups:
        for pi in grp:
            pr = pairs[pi]
            sls = [pr["a"]] + ([pr["b"]] if pr["b"] is not None else [])
            hmax = max(slabs[s]["h"] for s in sls)
            for k in range(hmax):
                for si in sls:
                    sl = slabs[si]
                    if k >= sl["h"]:
                        continue
                    push(dict(kind="e", slab=si, k=k, sn=sl["sn"]), sl["sn"])
            # the pair's nf columns ride the stream right behind its
            # layers: rows 0:64 = nf of A's slots, 64:128 = B's
            push(dict(kind="nf", pair=pi, sn=pr["pn"]), pr["pn"])
    E2 = eo
    return cls_list, slabs, pairs, groups, units, chunks, NSLOT, E2, PCOLS


def _prepare(node_feature, edge_state, edge_dst, W, b):
    node_feature = np.ascontiguousarray(np.asarray(node_feature), dtype=np.float32)
    edge_state = np.ascontiguousarray(np.asarray(edge_state), dtype=np.float32)
    edge_dst = np.asarray(edge_dst).astype(np.int64)
    W = np.ascontiguousarray(np.asarray(W), dtype=np.float32)
    b = np.asarray(b, dtype=np.float32).reshape(D, 1)

    N = node_feature.shape[0]
    eid_sorted = np.argsort(edge_dst, kind="stable")
    deg, degp, core_nodes = _classes_and_deal(edge_dst, N)
    starts = np.cumsum(deg) - deg
    cls_list, slabs, pairs, groups, units, chunks, NSLOT, E2, PCOLS = _plan(degp)

    # out_t rows 0:64 hold pair half A, 64:128 half B; out_slot maps
    # (half, pair col) -> node slot
    out_slot = np.full((2, PCOLS), -1, dtype=np.int64)
    for pr in pairs:
        a = slabs[pr["a"]]
        out_slot[0, pr["pcol0"] : pr["pcol0"] + a["sn"]] = a["col0"] + np.arange(
            a["sn"]
        )
        if pr["b"] is not None:
            bsl = slabs[pr["b"]]
            out_slot[1, pr["pcol0"] : pr["pcol0"] + bsl["sn"]] = bsl[
                "col0"
            ] + np.arange(bsl["sn"])

    es8 = edge_state.astype(E3M4)
    es8 = np.concatenate([es8, np.zeros((1, D), dtype=E3M4)], axis=0)
    nf8 = node_feature.astype(E3M4)

    in_maps = []
    col_node = np.full((N_CORES, NSLOT), -1, dtype=np.int64)
    W8 = W.astype(BF16)
    for c in range(N_CORES):
        nodes = np.asarray(core_nodes[c])
        ndeg = degp[nodes]
        ems = {}
        for ci, (d, n_g, so) in enumerate(cls_list):
            nodes_d = nodes[ndeg == d]
            kk = len(nodes_d)
            em = np.full((n_g, d), -1, dtype=np.int64)
            if kk:
                col = starts[nodes_d][:, None] + np.arange(d)[None, :]
                valid = np.arange(d)[None, :] < deg[nodes_d][:, None]
                em[:kk] = np.where(
                    valid, eid_sorted[np.where(valid, col, 0)], -1
                )
                col_node[c, so : so + kk] = nodes_d
            ems[ci] = em
        gidx = np.full((2, E2), -1, dtype=np.int64)
        nf_gidx = np.full((2, E2), -1, dtype=np.int64)  # -1 -> zero row
        run = 0
        for u in units:
            sn = u["sn"]
            if u["kind"] == "e":
                sl = slabs[u["slab"]]
                em = ems[sl["cls"]]
                so = cls_list[sl["cls"]][2]
                s = sl["col0"] - so
                k = u["k"]
                gidx[0, run : run + sn] = em[s : s + sn, 2 * k]
                gidx[1, run : run + sn] = em[s : s + sn, 2 * k + 1]
            else:
                pr = pairs[u["pair"]]
                for half, si in ((0, pr["a"]), (1, pr["b"])):
                    if si is None:
                        continue
                    sl = slabs[si]
                    nf_gidx[half, run : run + sl["sn"]] = col_node[
                        c, sl["col0"] : sl["col0"] + sl["sn"]
                    ]
            run += sn
        edge_t = np.empty((2 * D, E2), dtype=E3M4)
        edge_t[0:D] = es8[gidx[0]].T
        edge_t[D : 2 * D] = es8[gidx[1]].T
        nf8x = np.concatenate([nf8, np.zeros((1, D), dtype=E3M4)], axis=0)
        for half in range(2):
            sel = np.nonzero(nf_gidx[half] != -1)[0]
            edge_t[half * D : half * D + D, sel] = nf8x[nf_gidx[half][sel]].T
        consts = np.empty((128, 129), dtype=BF16)
        consts[0:64, 0:64] = W8[0:64]
        consts[64:128, 0:64] = W8[0:64]
        consts[0:64, 64:128] = W8[64:128]
        consts[64:128, 64:128] = W8[64:128]
        consts[0:64, 128] = b[:, 0].astype(BF16)
        consts[64:128, 128] = b[:, 0].astype(BF16)
        in_maps.append(
            {
                "edge_t": np.ascontiguousarray(edge_t),
                "consts": consts,
            }
        )
    return (
        in_maps,
        (cls_list, slabs, pairs, groups, units, chunks),
        NSLOT,
        E2,
        PCOLS,
        col_node,
        out_slot,
        N,
    )


def _install_shims():
    """Environment fixes: antenv.axon_hooks shim (NTFF profiling), no-op
    artifact upload, and a TileContext drain patch (this container's walrus
    rejects >1 sync-wait per instruction)."""
    try:
        import antenv.axon_hooks  # noqa: F401
    except ImportError:
        try:
            import antenv

            mod = types.ModuleType("antenv.axon_hooks")
            mod._hook = None

            def set_axon_ntff_profile_hook(h):
                mod._hook = h

            def get_axon_ntff_profile_hook():
                return mod._hook

            mod.set_axon_ntff_profile_hook = set_axon_ntff_profile_hook
            mod.get_axon_ntff_profile_hook = get_axon_ntff_profile_hook
            sys.modules["antenv.axon_hooks"] = mod
            antenv.axon_hooks = mod
            try:
                from trn_agent_boot.trn_boot import _ntff_profile_via_ctypes

                so = "/opt/axon/libaxon_pjrt.so"
                if os.path.exists(so):
                    set_axon_ntff_profile_hook(_ntff_profile_via_ctypes(so))
            except Exception:
                pass
        except Exception:
            pass
    try:
        import concourse.bass_utils as bu

        bu.upload_artifacts = lambda tmpdir: "local://" + tmpdir
    except Exception:
        pass
    import concourse.mybir as mybir
    import concourse.tile as tile_mod
    from concourse.vector_clock import ScopedClock

    if getattr(tile_mod.TileContext, "_drain_patched", False):
        return
    tile_mod.TileContext._orig_drain_and_barrier = (
        tile_mod.TileContext._drain_and_barrier
    )

    def _drain_and_barrier(self, tick_clock, wait_clock):
        nc = self.nc
        probe = nc.sync.nop(nofuse=True, hint="drain_wait_split")
        wait_clock.add_sem_waits(
            probe.ins, ScopedClock({None: tick_clock.global_clock})
        )
        waits = list(probe.ins.sync_info.on_wait)
        probe.ins.sync_info.on_wait = waits[:1]
        for w in waits[1:]:
            nop = nc.sync.nop(nofuse=True, hint="drain_wait_split")
            nop.ins.sync_info = mybir.SyncInfo(on_update=[], on_wait=[w])
        nc.sync.drain()
        nc.all_engine_barrier()
        assert self.sems is not None
        popped = nc._tile_sem_poison_stack.pop()
        assert popped is self._sem_poison
        nc.clear_and_free_semaphores(list(self.sems.allocated().values()))
        nc.all_engine_barrier()

    tile_mod.TileContext._drain_and_barrier = _drain_and_barrier
    tile_mod.TileContext._patched_drain_and_barrier = _drain_and_barrier
    tile_mod.TileContext._drain_patched = True


def _split_multiwaits(nc):
    """Walrus here allows at most ONE sync-wait per instruction: hoist extra
    waits onto preceding NoOps on the same engine."""
    import concourse.mybir as mybir

    for fn in nc.m.functions:
        for blk in fn.blocks:
            insts = blk.instructions
            new = []
            for ins in insts:
                si = getattr(ins, "sync_info", None)
                waits = list(si.on_wait) if si is not None and si.on_wait else []
                if len(waits) > 1:
                    for jw, w in enumerate(waits[:-1]):
                        nop = mybir.InstNoOp(
                            name=f"{ins.name}-wsplit{jw}",
                            engine=ins.engine,
                            bass_nofuse=True,
                            sync_info=mybir.SyncInfo(on_update=[], on_wait=[w]),
                        )
                        new.append(nop)
                    si.on_wait = [waits[-1]]
                new.append(ins)
            blk.instructions[:] = new


def _prebarrier_dma_hoist(nc, per_engine=2):
    """Move each HWDGE engine's first wait-free DMACopys to before its
    entry-barrier depart (EventSemaphore): the transfers then stream
    during the ~7us NEFF/Tile preamble instead of after it. DMAHW sems
    are NRT-zeroed at load, so pre-barrier +16 updates are safe."""
    import concourse.mybir as mybir

    for fn in nc.m.functions:
        seq = []  # (blk, idx, ins) in program order
        for blk in fn.blocks:
            for i, ins in enumerate(blk.instructions):
                seq.append((blk, i, ins))
        for eng in ("SP", "Activation"):
            bar = None
            moves = []
            for blk, i, ins in seq:
                if str(ins.engine).split(".")[-1] != eng:
                    continue
                if bar is None:
                    if isinstance(
                        ins, mybir.InstEventSemaphore
                    ) and ins.name.startswith("barrier_"):
                        bar = (blk, ins)
                    continue
                if len(moves) >= per_engine:
                    break
                si = getattr(ins, "sync_info", None)
                if isinstance(ins, mybir.InstDMACopy) and not (
                    si is not None and si.on_wait
                ):
                    moves.append((blk, ins))
            if bar is None:
                continue
            bblk, bins = bar
            for mblk, mins in moves:
                mblk.instructions.remove(mins)
                bi = bblk.instructions.index(bins)
                bblk.instructions.insert(bi, mins)


def _build(plan, NSLOT, E2, PCOLS):
    import concourse.bass as bass
    import concourse.mybir as mybir
    from concourse.tile import TileContext

    cls_list, slabs, pairs, groups, units, chunks = plan
    f32 = mybir.dt.float32
    bf16 = mybir.dt.bfloat16
    f8e3 = mybir.dt.float8e3
    nc = bass.Bass("TRN2", target_bir_lowering=False, debug=False)
    edge_t = nc.declare_dram_parameter("edge_t", [128, E2], f8e3, isOutput=False)
    cp = nc.declare_dram_parameter("consts", [128, 129], bf16, isOutput=False)
    out_t = nc.declare_dram_parameter("out_t", [128, PCOLS], bf16, isOutput=True)

    # units grouped by pair, preserving stream (interleaved) order
    pair_units = {}
    for u in units:
        pi = u["pair"] if u["kind"] == "nf" else slabs[u["slab"]]["pair"]
        pair_units.setdefault(pi, []).append(u)

    with TileContext(nc) as tc:
        with (
            tc.tile_pool(name="const", bufs=1) as cpool,
            tc.tile_pool(name="edges", bufs=len(chunks)) as epool,
            tc.tile_pool(name="psum", bufs=PSUM_BUFS, space="PSUM") as ppool,
            tc.tile_pool(name="outs", bufs=1) as opool,
        ):
            ob_all = opool.tile([128, PCOLS], bf16)
            # edge chunk 0 first in sync's queue: the PE's first real work
            ech0 = epool.tile(
                [128, chunks[0]["cols"]], f8e3, tag="eb0", name="ebuf0", bufs=1
            )
            nc.sync.dma_start(
                out=ech0[:], in_=edge_t[:, 0 : chunks[0]["cols"]]
            )
            cb = cpool.tile([128, 129], bf16)
            nc.scalar.dma_start(out=cb[:], in_=cp[:])
            w1 = cb[:, 0:64]
            w22 = cb[:, 64:128]
            bt = cb[:, 128:129]

            # hoist every chunk DMA to the program head: per-chunk tiles
            # have no input deps, and issuing early keeps the HWDGE rings
            # streaming instead of starving behind ReLU work on ACT
            ebufs = {0: ech0}  # chunk idx -> tile
            for cidx in range(1, len(chunks)):
                ch = chunks[cidx]
                t = epool.tile(
                    [128, ch["cols"]], f8e3, tag=f"eb{cidx}",
                    name=f"ebuf{cidx}", bufs=1,
                )
                eng = nc.sync if (cidx <= 1 or cidx % 2 == 0) else nc.scalar
                eng.dma_start(
                    out=t[:], in_=edge_t[:, ch["eo"] : ch["eo"] + ch["cols"]]
                )
                ebufs[cidx] = t

            def get_ebuf(cidx):
                return ebufs[cidx]

            def mm(ps, sl, rhs, lhsT, start, stop):
                half = sl["half"]
                nc.tensor.matmul(
                    out=ps[64 * half : 64 * half + 64, : sl["sn"]],
                    lhsT=lhsT, rhs=rhs, start=start, stop=stop,
                    tile_position=(0, 64 * half),
                )

            st_i = 0
            for grp in groups:
                pst = {}
                for pi in grp:
                    ps = ppool.tile(
                        [128, SLAB], f32, space="PSUM", tag="ps", name=f"ps{pi}"
                    )
                    pst[pi] = ps
                    for u in pair_units[pi]:
                        eb = get_ebuf(u["chunk"])
                        o = u["off"]
                        if u["kind"] == "e":
                            sl = slabs[u["slab"]]
                            mm(
                                ps, sl, eb[:, o : o + u["sn"]],
                                w22, u["k"] == 0, False,
                            )
                        else:
                            pr = pairs[u["pair"]]
                            for half, si in ((0, pr["a"]), (1, pr["b"])):
                                if si is None:
                                    continue
                                sl = slabs[si]
                                nc.tensor.matmul(
                                    out=ps[
                                        64 * half : 64 * half + 64, : sl["sn"]
                                    ],
                                    lhsT=w1[64 * half : 64 * half + 64, :],
                                    rhs=eb[
                                        64 * half : 64 * half + 64,
                                        o : o + sl["sn"],
                                    ],
                                    start=False, stop=True,
                                    tile_position=(64 * half, 64 * half),
                                )
                for pi in grp:
                    pr = pairs[pi]
                    ps = pst.pop(pi)
                    pn = pr["pn"]
                    prows = 128 if pr["b"] is not None else 64
                    pc0 = pr["pcol0"]
                    nc.scalar.activation(
                        out=ob_all[:prows, pc0 : pc0 + pn],
                        in_=ps[:prows, :pn],
                        func=mybir.ActivationFunctionType.Relu,
                        bias=bt[:prows, :],
                    )
                # store this group's finished region on alternating rings
                g0 = pairs[grp[0]]["pcol0"]
                ge = pairs[grp[-1]]["pcol0"] + pairs[grp[-1]]["pn"]
                eng = nc.scalar if st_i % 2 == 0 else nc.sync
                st_i += 1
                eng.dma_start(
                    out=out_t[:, g0:ge], in_=ob_all[:, g0:ge]
                )
    _split_multiwaits(nc)
    if os.environ.get("GNN_PREBAR", "0") == "1":
        _prebarrier_dma_hoist(nc)
    return nc


def kernel(node_feature, edge_state, edge_dst, W, b):
    global _last_exec_time_ns, _last_results
    _install_shims()
    from concourse.bass_utils import run_bass_kernel_spmd

    in_maps, plan, NSLOT, E2, PCOLS, col_node, out_slot, N = _prepare(
        node_feature, edge_state, edge_dst, W, b
    )
    nc = _build(plan, NSLOT, E2, PCOLS)
    trace = bool(os.environ.get("GNN_TRACE"))
    res = run_bass_kernel_spmd(
        nc, in_maps, core_ids=list(range(N_CORES)), trace=trace
    )
    _last_exec_time_ns = res.exec_time_ns
    _last_results = res
    out = np.zeros((N, D), dtype=np.float32)
    for c in range(N_CORES):
        ot = np.asarray(res.results[c]["out_t"]).astype(np.float32)
        for half in range(2):
            pm = out_slot[half] >= 0
            slots = out_slot[half][pm]
            nodes = col_node[c][slots]
            v2 = nodes >= 0
            out[nodes[v2]] = ot[64 * half : 64 * half + 64][:, pm][:, v2].T
    return out


def last_exec_time_ns():
    return _last_exec_time_ns


def last_results():
    return _last_results
